# revision 3
# baseline (speedup 1.0000x reference)
"""Trainium2 Bass kernel for nn_AbsoluteNeuralLayer.

Reference computation:
    classical = x @ classical_weights + classical_biases          # [B, DOUT]
    probs[j]  = |scan of circulant "rotations" applied to s0|[0]^2
    out       = tanh(classical + probs[None, :])

Key simplification: the scan state s0 is a constant vector, and every step
maps a constant vector to a constant vector scaled by cos(angle)
(s_new[i] = cos*s - sin*s + sin*s = cos*s elementwise).  Hence
    probs[j] = (prod_{t<48} cos(ang[j, t]))^2 / DIN
with ang[j, 3*d+g] = absolute_weights[d, j, g] for g < 3.

Sharding (8 cores): batch split 4 ways x dout split 2 ways.  Each core
computes out[1024 batch rows, 1024 dout cols]:
  - fp32r matmul (full-rate fp32 on the PE) with dout on PSUM partitions
    and batch on the free dim, accumulating over K=2048 in 16 k-tiles.
  - probs+bias computed once per core on ACT/DVE (tiny) and applied as the
    per-partition bias of the Tanh activation that drains PSUM.
Outputs are written transposed ([dout, batch] per core) and un-transposed
on the host during the gather.
"""

import math

import numpy as np

import concourse.bacc as bacc
import concourse.mybir as mybir
from concourse.tile import TileContext
from concourse.bass_utils import run_bass_kernel_spmd

B, DIN, DOUT, DEPTH = 4096, 2048, 2048, 16
NCORES = 8
BB, DB = 4, 2            # batch blocks x dout blocks (BB*DB == NCORES)
MB, NB = B // BB, DOUT // DB   # per-core batch rows (1024) / dout cols (1024)
KT = DIN // 128          # 16 contraction tiles
NT = NB // 128           # 8 dout tiles (PSUM partition groups)
MCH = 512                # batch (moving free dim) chunk, one PSUM bank of fp32
MC = MB // MCH           # 2 chunks
NANG = 3 * DEPTH         # 48 angles per output column

F32 = mybir.dt.float32
F32R = mybir.dt.float32r
AF = mybir.ActivationFunctionType

_NC_CACHE = None


def _build():
    nc = bacc.Bacc("TRN2", target_bir_lowering=False, debug=False, num_devices=NCORES)
    xt = nc.dram_tensor("xt", [DIN, MB], F32R, kind="ExternalInput")
    w = nc.dram_tensor("w", [DIN, NB], F32R, kind="ExternalInput")
    ang = nc.dram_tensor("ang", [128, NT * NANG], F32, kind="ExternalInput")
    bias = nc.dram_tensor("bias", [128, NT], F32, kind="ExternalInput")
    outT = nc.dram_tensor("outT", [NB, MB], F32, kind="ExternalOutput")

    with TileContext(nc) as tc:
        with (
            tc.tile_pool(name="big", bufs=1) as big,
            tc.tile_pool(name="small", bufs=1) as small,
            tc.tile_pool(name="outp", bufs=4) as outp,
            tc.tile_pool(name="psum", bufs=1, space="PSUM") as psump,
        ):
            # ---- probs + bias (tiny, runs early on ACT/DVE) ----
            ang_sb = small.tile([128, NT * NANG], F32, tag="ang")
            nc.sync.dma_start(out=ang_sb, in_=ang[:, :])
            bias_sb = small.tile([128, NT], F32, tag="bias")
            nc.sync.dma_start(out=bias_sb, in_=bias[:, :])
            halfpi = small.tile([128, 1], F32, tag="halfpi")
            nc.any.memset(halfpi, math.pi / 2)
            cos_sb = small.tile([128, NT * NANG], F32, tag="cos")
            nc.scalar.activation(cos_sb, ang_sb, AF.Sin, bias=halfpi)

            def v3(t):
                return t.rearrange("p (a b) -> p a b", a=NT)

            # product tree over the 48 cosines: 48->24->12->6->3->1
            t24 = small.tile([128, NT * 24], F32, tag="t24")
            nc.vector.tensor_mul(v3(t24), v3(cos_sb)[:, :, 0:24], v3(cos_sb)[:, :, 24:48])
            t12 = small.tile([128, NT * 12], F32, tag="t12")
            nc.vector.tensor_mul(v3(t12), v3(t24)[:, :, 0:12], v3(t24)[:, :, 12:24])
            t6 = small.tile([128, NT * 6], F32, tag="t6")
            nc.vector.tensor_mul(v3(t6), v3(t12)[:, :, 0:6], v3(t12)[:, :, 6:12])
            t3 = small.tile([128, NT * 3], F32, tag="t3")
            nc.vector.tensor_mul(v3(t3), v3(t6)[:, :, 0:3], v3(t6)[:, :, 3:6])
            t1 = small.tile([128, NT], F32, tag="t1")
            nc.vector.tensor_mul(v3(t1), v3(t3)[:, :, 0:1], v3(t3)[:, :, 1:2])
            nc.vector.tensor_mul(v3(t1), v3(t1), v3(t3)[:, :, 2:3])
            # probs = prod^2 / DIN, then fold in the classical bias
            sq = small.tile([128, NT], F32, tag="sq")
            nc.vector.tensor_mul(sq, t1, t1)
            nc.vector.tensor_scalar_mul(sq, sq, 1.0 / DIN)
            btot = small.tile([128, NT], F32, tag="btot")
            nc.vector.tensor_add(btot, sq, bias_sb)

            # ---- stream W (full) + xT chunk 0 ----
            w_sb = []
            x_sb = [[None] * KT for _ in range(MC)]
            for k in range(KT):
                wt = big.tile([128, NB], F32R, tag=f"w{k}")
                nc.sync.dma_start(out=wt, in_=w[128 * k:128 * (k + 1), :])
                w_sb.append(wt)
                x0 = big.tile([128, MCH], F32R, tag=f"x0_{k}")
                nc.sync.dma_start(out=x0, in_=xt[128 * k:128 * (k + 1), 0:MCH])
                x_sb[0][k] = x0

            # ---- pass A: m-chunk 0, k-outer over 8 concurrent PSUM groups ----
            ps = [
                psump.tile([128, MCH], F32, tag=f"ps{n}", name=f"psA{n}")
                for n in range(NT)
            ]
            for k in range(KT):
                for n in range(NT):
                    nc.tensor.matmul(
                        ps[n],
                        w_sb[k][:, 128 * n:128 * (n + 1)],
                        x_sb[0][k],
                        start=(k == 0),
                        stop=(k == KT - 1),
                    )

            # xT chunk 1 DMAs (overlap with pass A compute)
            for k in range(KT):
                x1 = big.tile([128, MCH], F32R, tag=f"x1_{k}")
                nc.sync.dma_start(out=x1, in_=xt[128 * k:128 * (k + 1), MCH:MB])
                x_sb[1][k] = x1

            # pass A epilogue: tanh(psum + btot[n]) -> SBUF -> DRAM
            for n in range(NT):
                o = outp.tile([128, MCH], F32, tag="o")
                nc.scalar.activation(o, ps[n], AF.Tanh, bias=btot[:, n:n + 1])
                nc.sync.dma_start(out=outT[128 * n:128 * (n + 1), 0:MCH], in_=o)

            # ---- pass B: m-chunk 1 ----
            ps = [
                psump.tile([128, MCH], F32, tag=f"ps{n}", name=f"psB{n}")
                for n in range(NT)
            ]
            for k in range(KT):
                for n in range(NT):
                    nc.tensor.matmul(
                        ps[n],
                        w_sb[k][:, 128 * n:128 * (n + 1)],
                        x_sb[1][k],
                        start=(k == 0),
                        stop=(k == KT - 1),
                    )
            for n in range(NT):
                o = outp.tile([128, MCH], F32, tag="o")
                nc.scalar.activation(o, ps[n], AF.Tanh, bias=btot[:, n:n + 1])
                nc.sync.dma_start(out=outT[128 * n:128 * (n + 1), MCH:MB], in_=o)

    nc.compile()
    return nc


def _get_nc():
    global _NC_CACHE
    if _NC_CACHE is None:
        _NC_CACHE = _build()
    return _NC_CACHE


def _in_map_for_core(core, x, absolute_weights, classical_weights, classical_biases):
    i, j = core % BB, core // BB
    rows = slice(i * MB, (i + 1) * MB)
    cols = slice(j * NB, (j + 1) * NB)
    xt = np.ascontiguousarray(x[rows, :].T)                       # [DIN, MB]
    w = np.ascontiguousarray(classical_weights[:, cols])          # [DIN, NB]
    # ang[j_local, 3*d+g] = absolute_weights[d, j, g]
    angj = np.transpose(absolute_weights[:, cols, :3], (1, 0, 2)).reshape(NB, NANG)
    # SBUF layout: ang_sb[p, t*48 + s] = angj[128*t + p, s]
    ang_sb = np.ascontiguousarray(
        angj.reshape(NT, 128, NANG).transpose(1, 0, 2).reshape(128, NT * NANG)
    )
    bias_sb = np.ascontiguousarray(classical_biases[cols].reshape(NT, 128).T)
    return {
        "xt": xt.astype(np.float32, copy=False),
        "w": w.astype(np.float32, copy=False),
        "ang": ang_sb.astype(np.float32, copy=False),
        "bias": bias_sb.astype(np.float32, copy=False),
    }


def kernel(x, absolute_weights, classical_weights, classical_biases, **_ignored):
    x = np.asarray(x, dtype=np.float32)
    absolute_weights = np.asarray(absolute_weights, dtype=np.float32)
    classical_weights = np.asarray(classical_weights, dtype=np.float32)
    classical_biases = np.asarray(classical_biases, dtype=np.float32)

    nc = _get_nc()
    in_maps = [
        _in_map_for_core(c, x, absolute_weights, classical_weights, classical_biases)
        for c in range(NCORES)
    ]
    res = run_bass_kernel_spmd(nc, in_maps, list(range(NCORES)))

    out = np.empty((B, DOUT), np.float32)
    for c in range(NCORES):
        i, j = c % BB, c // BB
        out[i * MB:(i + 1) * MB, j * NB:(j + 1) * NB] = res.results[c]["outT"].T
    return out
